# revision 1
# baseline (speedup 1.0000x reference)
"""Dense soft-MoE layer for Trainium2, expert-parallel across 8 NeuronCores.

Reference computation (T=4096 tokens, D=1024, F=4096, E=8 experts):
    gate = softmax(x @ gate_w + gate_b)                  # [T, E]
    h_e  = gelu(x @ w1[e] + b1[e])                       # [T, F]
    y_e  = h_e @ w2[e] + b2[e]                           # [T, D]
    out  = sum_e gate[:, e:e+1] * y_e                    # [T, D]

Sharding: expert-parallel — core e computes gate[:, e] * y_e for all
tokens; the host sums the 8 partial outputs. Everything on device runs
in a transposed layout (hT[f, t], yT[d, t]) so no on-device transposes
are needed: every matmul's stationary operand is a weight block and the
moving operand streams 512 tokens (N=512, one PSUM bank).

Matmul data is fp16 (cast on host) with fp32 PSUM accumulation; the
expert gate is computed on-device in a [E, tokens] layout and applied
via a rank-1 broadcast matmul. Per-core expert selection is data-driven
(a one-hot selector input) so all 8 cores run the same SPMD program.
"""
import sys

sys.path.insert(0, "/opt/trn_rl_repo")

import numpy as np

D = 1024
F = 4096
E = 8
T = 4096
P = 128
TC = 512            # token chunk
NCH = T // TC       # 8 chunks
KD = D // P         # 8 d-tiles (contraction of first matmul)
KF = F // P         # 32 f-tiles (contraction of second matmul)
ND = D // P         # 8 output d-tiles

_cache = {}


def _build(reps: int = 1, loop_n: int = 0, use_tanh: bool = True, gate_pool: bool = True):
    import contextlib
    import concourse.mybir as mybir
    import concourse.tile as tile
    from concourse import bacc

    dt = mybir.dt
    AF = mybir.ActivationFunctionType
    ALU = mybir.AluOpType

    nc = bacc.Bacc(None, target_bir_lowering=False, debug=False)

    xT = nc.dram_tensor("xT", [D, T], dt.float16, kind="ExternalInput")
    w1e = nc.dram_tensor("w1e", [D, F], dt.float16, kind="ExternalInput")
    w2e = nc.dram_tensor("w2e", [F, D], dt.float16, kind="ExternalInput")
    b1e = nc.dram_tensor("b1e", [F], dt.float32, kind="ExternalInput")
    b2e = nc.dram_tensor("b2e", [D], dt.float32, kind="ExternalInput")
    gw = nc.dram_tensor("gw", [D, E], dt.float16, kind="ExternalInput")
    # gbh holds gate_b / 2: the gate exp() is computed via tanh so it shares
    # the ACT gelu table (no per-chunk table reloads): e^x = (1+t)/(1-t),
    # t = tanh(x/2) = tanh(logits*0.5 + gate_b*0.5)
    gbh = nc.dram_tensor("gbh", [E, 1], dt.float32, kind="ExternalInput")
    # one-hot selector for this core's expert (keeps the program SPMD)
    sele = nc.dram_tensor("sele", [E, 1], dt.float16, kind="ExternalInput")
    outT = nc.dram_tensor("outT", [D, T], dt.float32, kind="ExternalOutput")

    with tile.TileContext(nc) as tc:
        with tc.tile_pool(name="weights", bufs=1) as wpool, \
             tc.tile_pool(name="consts", bufs=1) as cpool, \
             tc.tile_pool(name="xin", bufs=2) as xpool, \
             tc.tile_pool(name="hbuf", bufs=1) as hpool, \
             tc.tile_pool(name="psum", bufs=6, space="PSUM") as ppool, \
             tc.tile_pool(name="gpsum", bufs=2, space="PSUM") as gpsum, \
             tc.tile_pool(name="small", bufs=4) as spool, \
             tc.tile_pool(name="gate", bufs=2) as gatepool, \
             tc.tile_pool(name="outb", bufs=3) as opool:

            w1_re = w1e.rearrange("(k p) f -> p k f", p=P)
            w1_sb = wpool.tile([P, KD, F], dt.float16)
            for f8 in range(8):
                fs = slice(f8 * (F // 8), (f8 + 1) * (F // 8))
                nc.sync.dma_start(w1_sb[:, :, fs], w1_re[:, :, fs])
            w2_re = w2e.rearrange("(k p) d -> p k d", p=P)
            w2_sb = wpool.tile([P, KF, D], dt.float16)
            for k8 in range(4):
                ks = slice(k8 * (KF // 4), (k8 + 1) * (KF // 4))
                nc.sync.dma_start(w2_sb[:, ks, :], w2_re[:, ks, :])

            b1_sb = cpool.tile([P, KF], dt.float32)
            nc.sync.dma_start(b1_sb[:], b1e.rearrange("(f p) -> p f", p=P))
            b2_sb = cpool.tile([P, ND], dt.float32)
            nc.sync.dma_start(b2_sb[:], b2e.rearrange("(d p) -> p d", p=P))
            gw_sb = cpool.tile([P, KD, E], dt.float16)
            nc.sync.dma_start(gw_sb[:], gw.rearrange("(k p) e -> p k e", p=P))
            gbh_sb = cpool.tile([E, 1], dt.float32)
            nc.sync.dma_start(gbh_sb[:], gbh[:])
            sele_sb = cpool.tile([E, 1], dt.float16)
            nc.sync.dma_start(sele_sb[:], sele[:])
            gbf_sb = cpool.tile([E, 1], dt.float32)
            nc.vector.tensor_scalar_mul(gbf_sb[:], gbh_sb[:], 2.0)
            ones8 = cpool.tile([E, 1], dt.float16)
            nc.any.memset(ones8[:], 1.0)
            ones1 = cpool.tile([1, P], dt.float16)
            nc.any.memset(ones1[:], 1.0)

            xT_re = xT.rearrange("(k p) t -> p k t", p=P)

            loop_cm = tc.For_i(0, loop_n, 1) if loop_n else contextlib.nullcontext()
            with loop_cm:
              for _rep in range(reps):
                for c in range(NCH):
                    tsl = slice(c * TC, (c + 1) * TC)
                    x_sb = xpool.tile([P, KD, TC], dt.float16, tag="x")
                    nc.sync.dma_start(x_sb[:], xT_re[:, :, tsl])

                    # --- gate: gcol[1, TC] = softmax(x@gw+gb)[:, e]^T ---
                    gp = gpsum if gate_pool else ppool
                    gtag = "gmm" if gate_pool else "mm"
                    lg = gp.tile([E, TC], dt.float32, tag=gtag)
                    for k in range(KD):
                        nc.tensor.matmul(lg[:], gw_sb[:, k, :], x_sb[:, k, :],
                                         start=(k == 0), stop=(k == KD - 1))
                    expT = spool.tile([E, TC], dt.float16, tag="expT")
                    if use_tanh:
                        tt = spool.tile([E, TC], dt.float32, tag="gs")
                        nc.scalar.activation(tt[:], lg[:], AF.Tanh,
                                             bias=gbh_sb[:], scale=0.5)
                        bm = spool.tile([E, TC], dt.float32, tag="gs")
                        nc.vector.tensor_scalar(bm[:], tt[:], -1.0, 1.0,
                                                op0=ALU.mult, op1=ALU.add)
                        rb = spool.tile([E, TC], dt.float32, tag="gs")
                        nc.vector.reciprocal(rb[:], bm[:])
                        ap1 = spool.tile([E, TC], dt.float32, tag="gs")
                        nc.vector.tensor_scalar_add(ap1[:], tt[:], 1.0)
                        nc.vector.tensor_mul(expT[:], ap1[:], rb[:])
                    else:
                        nc.scalar.activation(expT[:], lg[:], AF.Exp,
                                             bias=gbf_sb[:])
                    den = gp.tile([1, TC], dt.float32, tag=gtag)
                    nc.tensor.matmul(den[:], ones8[:], expT[:], start=True, stop=True)
                    num = gp.tile([1, TC], dt.float32, tag=gtag)
                    nc.tensor.matmul(num[:], sele_sb[:], expT[:], start=True, stop=True)
                    rec = spool.tile([1, TC], dt.float32, tag="gs")
                    nc.vector.reciprocal(rec[:], den[:])
                    gcol = spool.tile([1, TC], dt.float16, tag="gcol")
                    nc.vector.tensor_mul(gcol[:], num[:], rec[:])
                    gbc = gp.tile([P, TC], dt.float32, tag=gtag)
                    nc.tensor.matmul(gbc[:], ones1[:], gcol[:], start=True, stop=True)
                    gate_sb = gatepool.tile([P, TC], dt.float32, tag="gate")
                    nc.vector.tensor_copy(gate_sb[:], gbc[:])

                    # --- hT[f, t] = gelu(w1^T x^T + b1) ---
                    hT = hpool.tile([P, KF, TC], dt.float16, tag="hT")
                    for f in range(KF):
                        ph = ppool.tile([P, TC], dt.float32, tag="mm")
                        for k in range(KD):
                            nc.tensor.matmul(ph[:], w1_sb[:, k, f * P:(f + 1) * P],
                                             x_sb[:, k, :],
                                             start=(k == 0), stop=(k == KD - 1))
                        nc.scalar.activation(hT[:, f, :], ph[:], AF.Gelu,
                                             bias=b1_sb[:, f:f + 1])

                    # --- yT[d, t] = w2^T hT ; out = gate * (yT + b2) ---
                    for d in range(ND):
                        py = ppool.tile([P, TC], dt.float32, tag="mm")
                        for f in range(KF):
                            nc.tensor.matmul(py[:], w2_sb[:, f, d * P:(d + 1) * P],
                                             hT[:, f, :],
                                             start=(f == 0), stop=(f == KF - 1))
                        ob = opool.tile([P, TC], dt.float32, tag="ob")
                        nc.vector.scalar_tensor_tensor(
                            ob[:], py[:], b2_sb[:, d:d + 1], gate_sb[:],
                            op0=ALU.add, op1=ALU.mult)
                        nc.sync.dma_start(outT[d * P:(d + 1) * P, tsl], ob[:])

    nc.compile()
    return nc


def kernel(inputs, gate_w, gate_b, w1, b1, w2, b2):
    from concourse.bass_utils import run_bass_kernel_spmd

    if "nc" not in _cache:
        _cache["nc"] = _build()
    nc = _cache["nc"]

    B, S, Dm = inputs.shape
    x = np.ascontiguousarray(inputs.reshape(-1, Dm))          # [T, D]
    xT16 = np.ascontiguousarray(x.T).astype(np.float16)       # [D, T]
    gw16 = np.asarray(gate_w, dtype=np.float16)
    gbh32 = np.asarray(gate_b, dtype=np.float32).reshape(E, 1) * 0.5

    in_maps = []
    for e in range(E):
        sele = np.zeros((E, 1), dtype=np.float16)
        sele[e, 0] = 1.0
        in_maps.append({
            "xT": xT16,
            "w1e": np.ascontiguousarray(w1[e]).astype(np.float16),
            "w2e": np.ascontiguousarray(w2[e]).astype(np.float16),
            "b1e": np.asarray(b1[e], dtype=np.float32),
            "b2e": np.asarray(b2[e], dtype=np.float32),
            "gw": gw16,
            "gbh": gbh32,
            "sele": sele,
        })

    res = run_bass_kernel_spmd(nc, in_maps, core_ids=list(range(E)))
    _cache["last_results"] = res

    acc = res.results[0]["outT"].astype(np.float64)
    for e in range(1, E):
        acc += res.results[e]["outT"]
    out = acc.T.astype(np.float32).reshape(B, S, Dm)
    return out



# revision 2
# speedup vs baseline: 1.0927x; 1.0927x over previous
"""Dense soft-MoE layer for Trainium2, expert-parallel across 8 NeuronCores.

V4: second matmul runs "flipped" — stationary = hT[f, tok-block],
moving = w2[f, d-half] — so each loaded stationary serves 2 matmuls
(the two d-halves). That halves LDWEIGHTS stationary switches in L2
(~100 ns each on this toolchain, walrus runs with FWL/ldw-opt off).
The L2 output lands in [token, d] layout, so the gate is applied as a
per-partition scalar (no broadcast matmul) and b2 is added inside the
accumulation group via a ones-stationary rank-1 matmul. The gate
softmax chain is software-pipelined one chunk ahead of the PE stream
(as in V2). Matmul operands are bf16; partial outputs are stored fp16
as [T, D] (natural layout) and summed across cores on the host.
"""
import sys

sys.path.insert(0, "/opt/trn_rl_repo")

import numpy as np
import ml_dtypes

D = 1024
F = 4096
E = 8
T = 4096
P = 128
TC = 512            # token chunk
NCH = T // TC       # 8 chunks
KD = D // P         # 8 k-tiles (contraction of first matmul)
KF = F // P         # 32 f-tiles (contraction of second matmul)
ND = D // P         # 8 output d-tiles
NTB = TC // P       # 4 token blocks per chunk
NDH = D // 512      # 2 d-halves

_cache = {}


def _build(reps: int = 1, loop_n: int = 0):
    import contextlib
    import concourse.mybir as mybir
    import concourse.tile as tile
    from concourse import bacc

    dt = mybir.dt
    AF = mybir.ActivationFunctionType
    ALU = mybir.AluOpType

    nc = bacc.Bacc(None, target_bir_lowering=False, debug=False)

    xT = nc.dram_tensor("xT", [D, T], dt.bfloat16, kind="ExternalInput")
    w1e = nc.dram_tensor("w1e", [D, F], dt.bfloat16, kind="ExternalInput")
    w2e = nc.dram_tensor("w2e", [F, D], dt.bfloat16, kind="ExternalInput")
    b1e = nc.dram_tensor("b1e", [F], dt.float32, kind="ExternalInput")
    b2r = nc.dram_tensor("b2r", [1, D], dt.bfloat16, kind="ExternalInput")
    gw = nc.dram_tensor("gw", [D, E], dt.bfloat16, kind="ExternalInput")
    # gbh holds gate_b / 2: the gate exp() is computed via tanh so it shares
    # the ACT gelu table: e^x = (1+t)/(1-t), t = tanh(x*0.5 + gate_b*0.5)
    gbh = nc.dram_tensor("gbh", [E, 1], dt.float32, kind="ExternalInput")
    # one-hot selector for this core's expert (keeps the program SPMD)
    sele = nc.dram_tensor("sele", [E, 1], dt.bfloat16, kind="ExternalInput")
    out = nc.dram_tensor("out", [T, D], dt.float16, kind="ExternalOutput")

    with tile.TileContext(nc) as tc:
        with tc.tile_pool(name="weights", bufs=1) as wpool, \
             tc.tile_pool(name="consts", bufs=1) as cpool, \
             tc.tile_pool(name="xin", bufs=2) as xpool, \
             tc.tile_pool(name="hbuf", bufs=1) as hpool, \
             tc.tile_pool(name="psum", bufs=2, space="PSUM") as ppool, \
             tc.tile_pool(name="py", bufs=2, space="PSUM") as pypool, \
             tc.tile_pool(name="glg", bufs=1, space="PSUM") as lgpool, \
             tc.tile_pool(name="gdn", bufs=1, space="PSUM") as dnpool, \
             tc.tile_pool(name="ggt", bufs=1, space="PSUM") as gtpool, \
             tc.tile_pool(name="small", bufs=4) as spool, \
             tc.tile_pool(name="small2", bufs=3) as s2pool, \
             tc.tile_pool(name="gate", bufs=2) as gatepool, \
             tc.tile_pool(name="outb", bufs=3) as opool:

            w1_re = w1e.rearrange("(k p) f -> p k f", p=P)
            w1_sb = wpool.tile([P, KD, F], dt.bfloat16)
            for f8 in range(8):
                fs = slice(f8 * (F // 8), (f8 + 1) * (F // 8))
                nc.sync.dma_start(w1_sb[:, :, fs], w1_re[:, :, fs])
            w2_re = w2e.rearrange("(k p) d -> p k d", p=P)
            w2_sb = wpool.tile([P, KF, D], dt.bfloat16)
            for k8 in range(4):
                ks = slice(k8 * (KF // 4), (k8 + 1) * (KF // 4))
                nc.sync.dma_start(w2_sb[:, ks, :], w2_re[:, ks, :])

            b1_sb = cpool.tile([P, KF], dt.float32)
            nc.sync.dma_start(b1_sb[:], b1e.rearrange("(f p) -> p f", p=P))
            b2_sb = cpool.tile([1, D], dt.bfloat16)
            nc.sync.dma_start(b2_sb[:], b2r[:])
            gw_sb = cpool.tile([P, KD, E], dt.bfloat16)
            nc.sync.dma_start(gw_sb[:], gw.rearrange("(k p) e -> p k e", p=P))
            gbh_sb = cpool.tile([E, 1], dt.float32)
            nc.sync.dma_start(gbh_sb[:], gbh[:])
            sele_sb = cpool.tile([E, 1], dt.bfloat16)
            nc.sync.dma_start(sele_sb[:], sele[:])
            ones8 = cpool.tile([E, 1], dt.bfloat16)
            nc.any.memset(ones8[:], 1.0)
            onesr = cpool.tile([1, P], dt.bfloat16)
            nc.any.memset(onesr[:], 1.0)
            ones11 = cpool.tile([1, 1], dt.bfloat16)
            nc.any.memset(ones11[:], 1.0)

            xT_re = xT.rearrange("(k p) t -> p k t", p=P)

            def emit_lg(c, x_sb):
                """gate logit matmuls for chunk c: lg[E, TC] (PSUM)."""
                lg = lgpool.tile([E, TC], dt.float32, tag="lg")
                for k in range(KD):
                    nc.tensor.matmul(lg[:], gw_sb[:, k, :], x_sb[:, k, :],
                                     start=(k == 0), stop=(k == KD - 1))
                return lg

            def emit_exp(lg):
                """ACT/DVE chain: expT[E, TC] = exp(lg + gate_b) via tanh."""
                expT = s2pool.tile([E, TC], dt.bfloat16, tag="expT")
                tt = spool.tile([E, TC], dt.float32, tag="gs")
                nc.scalar.activation(tt[:], lg[:], AF.Tanh,
                                     bias=gbh_sb[:], scale=0.5)
                bm = spool.tile([E, TC], dt.float32, tag="gs")
                nc.vector.tensor_scalar(bm[:], tt[:], -1.0, 1.0,
                                        op0=ALU.mult, op1=ALU.add)
                rb = spool.tile([E, TC], dt.float32, tag="gs")
                nc.vector.reciprocal(rb[:], bm[:])
                ap1 = spool.tile([E, TC], dt.float32, tag="gs")
                nc.vector.tensor_scalar_add(ap1[:], tt[:], 1.0)
                nc.vector.tensor_mul(expT[:], ap1[:], rb[:])
                return expT

            def emit_dennum(expT):
                """den = sum_e expT, num = expT[sel]; gcol = num/den."""
                den = dnpool.tile([1, TC], dt.float32, tag="den")
                nc.tensor.matmul(den[:], ones8[:], expT[:], start=True, stop=True)
                num = dnpool.tile([1, TC], dt.float32, tag="num")
                nc.tensor.matmul(num[:], sele_sb[:], expT[:], start=True, stop=True)
                rec = s2pool.tile([1, TC], dt.float32, tag="rec")
                nc.vector.reciprocal(rec[:], den[:])
                gcol = s2pool.tile([1, TC], dt.bfloat16, tag="gcol")
                nc.vector.tensor_mul(gcol[:], num[:], rec[:])
                return gcol

            def emit_gT(gcol):
                """transpose gcol[1, TC] -> gT_sb[P, NTB] (col tb = gate of
                token block tb), via NTB tiny rank-1 transpose matmuls."""
                gT_sb = gatepool.tile([P, NTB], dt.float32, tag="gT")
                for tb in range(NTB):
                    gt = gtpool.tile([P, 1], dt.float32, tag="gt")
                    nc.tensor.matmul(gt[:], gcol[0:1, tb * P:(tb + 1) * P],
                                     ones11[:], start=True, stop=True)
                    nc.vector.tensor_copy(gT_sb[:, tb:tb + 1], gt[:])
                return gT_sb

            def emit_l1(c, x_sb, hT, f0, f1):
                for f in range(f0, f1):
                    ph = ppool.tile([P, TC], dt.float32, tag="mm")
                    for k in range(KD):
                        nc.tensor.matmul(ph[:], w1_sb[:, k, f * P:(f + 1) * P],
                                         x_sb[:, k, :],
                                         start=(k == 0), stop=(k == KD - 1))
                    nc.scalar.activation(hT[:, f, :], ph[:], AF.Gelu,
                                         bias=b1_sb[:, f:f + 1])

            loop_cm = tc.For_i(0, loop_n, 1) if loop_n else contextlib.nullcontext()
            with loop_cm:
              for _rep in range(reps):
                state = {}
                for c in range(NCH):
                    tsl = slice(c * TC, (c + 1) * TC)
                    if c == 0:
                        x_sb = xpool.tile([P, KD, TC], dt.bfloat16, tag="x")
                        nc.sync.dma_start(x_sb[:], xT_re[:, :, tsl])
                        state[0] = x_sb
                    x_sb = state[c]
                    hT = hpool.tile([P, KF, TC], dt.bfloat16, tag="hT")

                    if c == 0:
                        # self-contained body: gate(0) boots here, with a
                        # short L1 filler hiding the ACT/DVE latency
                        lg0 = emit_lg(0, x_sb)
                        exp0 = emit_exp(lg0)
                        emit_l1(c, x_sb, hT, 0, 8)
                        gcol = emit_dennum(exp0)
                        emit_l1(c, x_sb, hT, 8, 16)
                    else:
                        gcol = state[(c, "gcol")]
                        emit_l1(c, x_sb, hT, 0, 16)

                    # prefetch + gate pipeline for chunk c+1
                    if c + 1 < NCH:
                        nsl = slice((c + 1) * TC, (c + 2) * TC)
                        xn = xpool.tile([P, KD, TC], dt.bfloat16, tag="x")
                        nc.sync.dma_start(xn[:], xT_re[:, :, nsl])
                        state[c + 1] = xn
                        lgn = emit_lg(c + 1, xn)
                        expn = emit_exp(lgn)

                    emit_l1(c, x_sb, hT, 16, KF)

                    if c + 1 < NCH:
                        state[(c + 1, "gcol")] = emit_dennum(expn)

                    gT_sb = emit_gT(gcol)

                    # --- L2 flipped: y[tok, d] = sum_f h[f, tok]^T w2[f, d]
                    #     + b2 (rank-1 ones matmul), gated per-partition ---
                    for tb in range(NTB):
                        pys = []
                        for _dh in range(NDH):
                            pyt = pypool.tile([P, 512], dt.float32, tag="py")
                            pys.append(pyt)
                        for f in range(KF):
                            hsl = hT[:, f, tb * P:(tb + 1) * P]
                            for dh in range(NDH):
                                nc.tensor.matmul(
                                    pys[dh][:], hsl,
                                    w2_sb[:, f, dh * 512:(dh + 1) * 512],
                                    start=(f == 0), stop=False)
                        for dh in range(NDH):
                            nc.tensor.matmul(
                                pys[dh][:], onesr[:],
                                b2_sb[0:1, dh * 512:(dh + 1) * 512],
                                start=False, stop=True)
                        for dh in range(NDH):
                            ob = opool.tile([P, 512], dt.float16, tag="ob")
                            nc.vector.tensor_scalar_mul(
                                ob[:], pys[dh][:], gT_sb[:, tb:tb + 1])
                            nc.scalar.dma_start(
                                out[c * TC + tb * P:c * TC + (tb + 1) * P,
                                    dh * 512:(dh + 1) * 512], ob[:])

    nc.compile()
    return nc


def make_in_maps(inputs, gate_w, gate_b, w1, b1, w2, b2):
    x = np.ascontiguousarray(np.asarray(inputs).reshape(-1, D))       # [T, D]
    xT16 = np.ascontiguousarray(x.T).astype(ml_dtypes.bfloat16)       # [D, T]
    gw16 = np.asarray(gate_w, dtype=ml_dtypes.bfloat16)
    gbh32 = np.asarray(gate_b, dtype=np.float32).reshape(E, 1) * 0.5

    in_maps = []
    for e in range(E):
        sele = np.zeros((E, 1), dtype=ml_dtypes.bfloat16)
        sele[e, 0] = 1.0
        in_maps.append({
            "xT": xT16,
            "w1e": np.ascontiguousarray(w1[e]).astype(ml_dtypes.bfloat16),
            "w2e": np.ascontiguousarray(w2[e]).astype(ml_dtypes.bfloat16),
            "b1e": np.asarray(b1[e], dtype=np.float32),
            "b2r": np.asarray(b2[e], dtype=ml_dtypes.bfloat16).reshape(1, D),
            "gw": gw16,
            "gbh": gbh32,
            "sele": sele,
        })
    return in_maps


def make_in_map(inputs_dict, e):
    return make_in_maps(inputs_dict["inputs"], inputs_dict["gate_w"],
                        inputs_dict["gate_b"], inputs_dict["w1"],
                        inputs_dict["b1"], inputs_dict["w2"],
                        inputs_dict["b2"])[e]


def kernel(inputs, gate_w, gate_b, w1, b1, w2, b2):
    from concourse.bass_utils import run_bass_kernel_spmd

    if "nc" not in _cache:
        _cache["nc"] = _build()
    nc = _cache["nc"]

    B, S, Dm = inputs.shape
    in_maps = make_in_maps(inputs, gate_w, gate_b, w1, b1, w2, b2)

    res = run_bass_kernel_spmd(nc, in_maps, core_ids=list(range(E)))
    _cache["last_results"] = res

    acc = res.results[0]["out"].astype(np.float64)
    for e in range(1, E):
        acc += res.results[e]["out"]
    return acc.astype(np.float32).reshape(B, S, Dm)
